# revision 3
# baseline (speedup 1.0000x reference)
"""DeformConv3D on 8 TRN2 cores — SINGLE fused launch.

Per core (h-band of 12 output rows):
  P1': offset conv computed directly in the torch-contiguous-view
       scrambled arrangement this core's gather needs: for each target
       depth l, 3 fixed segments (15/12/15 source rows) select channel
       group j=(3l+s)//16 and source depth l'=(3l+s)%16 via
       host-permuted input slabs (xa) + host-sliced weights (wsched).
       Output = flat [42*96] per plane; component k of the gather
       coords is the stride-3 slice [3n+k].
  P2: clamp coords on-device, separable 5-tap tent-weight gather (DVE)
  P3: main 3x3x3 conv + bias on the 12-row band (bf16 matmuls)
No host round-trip, no collectives: one launch instead of two.
"""
import sys
import numpy as np

sys.path.insert(0, "/opt/trn_rl_repo")
from concourse import bass, bacc, tile, mybir
from concourse.bass_utils import run_bass_kernel_spmd

F32 = mybir.dt.float32
BF16 = mybir.dt.bfloat16
F16 = mybir.dt.float16
ALU = mybir.AluOpType
AF = mybir.ActivationFunctionType

B, C, L, H, W = 2, 64, 16, 96, 96
NCORES = 8
HB = 12                 # output rows per core
HG = 14                 # band rows (12k-1 .. 12k+12)
NPP = HG * W            # 1344
TAPS = (-2, -1, 0, 1, 2)
SEGS = [(0, 5), (5, 4), (9, 5)]          # (band-row start, n band rows)
SEG_RBASE = (0, 17, 31)                  # xa row offset per segment
SEG_COL0 = (0, 1440, 2592)               # src column offset per segment
# P1' psum chunks per segment: (chunk src-row start, n src rows)
SEG_CHUNKS = ([(0, 5), (5, 5), (10, 5)],
              [(0, 4), (4, 4), (8, 4)],
              [(0, 5), (5, 5), (10, 5)])
LCH = 2                 # target-l chunk size for the gather window

_nc_cache = None


def core_sched(k):
    rows = [12 * k - 1 + i for i in range(HG)]
    sm = []
    for h in rows:
        s = 0 if h < 0 else (2 if h > 95 else h // 32)
        sm.append((s, h - 32 * s))
    sched = []
    for (i0, nr) in SEGS:
        s, m32_0 = sm[i0]
        for t in range(nr):
            assert sm[i0 + t] == (s, m32_0 + t)
        sched.append((s, 3 * m32_0))
    return sched


def build_fused():
    nc = bacc.Bacc("TRN2", target_bir_lowering=False, debug=False,
                   num_devices=NCORES)
    xwin = nc.dram_tensor("xwin", [B, C, L, 20, W], F16,
                          kind="ExternalInput").ap()
    xa = nc.dram_tensor("xa", [B, C, L, 3, 48, 98], BF16,
                        kind="ExternalInput").ap()
    wsched = nc.dram_tensor("wsched", [128, L, 3, 27, 64], BF16,
                            kind="ExternalInput").ap()
    wct_in = nc.dram_tensor("wct", [128, 27, 64], F16,
                            kind="ExternalInput").ap()
    bct_in = nc.dram_tensor("bct", [64, 1], F32, kind="ExternalInput").ap()
    bnds_in = nc.dram_tensor("bnds", [128, 5, NPP], F16,
                             kind="ExternalInput").ap()
    out_ext = nc.dram_tensor("out", [B, 64, L, HB, W], F32,
                             kind="ExternalOutput").ap()

    # const APs for activation biases (-t for tent taps); 0.0/1.0 built in
    for v in (2.0, -1.0, -2.0):
        t_ = nc.alloc_sbuf_tensor(f"cstb{int(v*10)}", [128, 1], F32)
        nc.gpsimd.memset(t_.ap(), v)
        nc.const_aps.aps[(F32, v)] = t_.ap()
    nc.all_engine_barrier()

    with tile.TileContext(nc) as tc:
        with tc.tile_pool(name="pp", bufs=1) as pp, \
             tc.tile_pool(name="pa", bufs=2) as pa, \
             tc.tile_pool(name="pc", bufs=1) as pc, \
             tc.tile_pool(name="psp", bufs=3, space="PSUM") as psp:
            bndst = pp.tile([128, 5, NPP], F16)
            nc.sync.dma_start(bndst[:], bnds_in)
            win = pp.tile([128, 20, LCH + 4, 100], F16)
            wctt = pp.tile([128, 27, 64], F16)
            nc.sync.dma_start(wctt[:], wct_in)
            bctt = pp.tile([64, 1], F32)
            nc.sync.dma_start(bctt[:], bct_in)
            accb_t = {}

            def emit_p3(lp):
                taps = [t for t in range(27)
                        if 0 <= lp + t // 9 - 1 <= 15]
                for b in range(B):
                    for hc0, hcn in ((0, 5), (5, 5), (10, 2)):
                        nmm = hcn * 96
                        ps = psp.tile([64, 480], F32, tag="ps3")
                        for ti, t in enumerate(taps):
                            dz, rem = divmod(t, 9)
                            dy, dx = divmod(rem, 3)
                            rhs = accb_t[lp + dz - 1][
                                64 * b:64 * b + 64,
                                hc0 + dy:hc0 + dy + hcn, dx:dx + 96]
                            nc.tensor.matmul(
                                ps[:, :nmm],
                                wctt[64 * b:64 * b + 64, t, :], rhs,
                                start=(ti == 0), stop=(ti == len(taps) - 1))
                        ob = pc.tile([64, 480], F32, tag="ob")
                        nc.scalar.activation(ob[:, :nmm], ps[:, :nmm],
                                             AF.Identity, bias=bctt[:],
                                             scale=1.0)
                        nc.sync.dma_start(
                            out_ext[b, :, lp, hc0:hc0 + hcn, :]
                            .rearrange("m h x -> m (h x)"),
                            ob[:, :nmm])

            for lc0 in range(0, L, LCH):
                # (re)load gather window depths lc0-2 .. lc0+LCH+1
                nc.vector.memset(
                    win[:].rearrange("p y z x -> p (y z x)"), 0.0)
                for b in range(B):
                    for z in range(lc0 - 2, lc0 + LCH + 2):
                        if 0 <= z < L:
                            nc.sync.dma_start(
                                win[64 * b:64 * b + 64, :, z - lc0 + 2, 2:98],
                                xwin[b, :, z])
                for l in range(lc0, lc0 + LCH):
                    # ---- P1': scrambled offset conv ----
                    xat = pa.tile([128, 3, 48, 98], BF16, tag="xa")
                    for b in range(B):
                        nc.sync.dma_start(xat[64 * b:64 * b + 64], xa[b, :, l])
                    wst = pa.tile([128, 3, 27, 64], BF16, tag="wst")
                    nc.sync.dma_start(wst[:], wsched[:, l])
                    src = pa.tile([128, 4032], F16, tag="src")
                    for b in range(B):
                        for seg in range(3):
                            rb = SEG_RBASE[seg]
                            for (cr0, cn) in SEG_CHUNKS[seg]:
                                nmm = cn * 96
                                ps = psp.tile([64, 480], F32, tag="ps1")
                                for t in range(27):
                                    dz, rem = divmod(t, 9)
                                    dy, dx = divmod(rem, 3)
                                    rhs = xat[64 * b:64 * b + 64, dz,
                                              rb + cr0 + dy:rb + cr0 + dy + cn,
                                              dx:dx + 96]
                                    nc.tensor.matmul(
                                        ps[:, :nmm], wst[64 * b:64 * b + 64, seg, t, :], rhs,
                                        start=(t == 0), stop=(t == 26))
                                col0 = SEG_COL0[seg] + cr0 * 96
                                nc.scalar.activation(
                                    src[64 * b:64 * b + 64,
                                        col0:col0 + nmm],
                                    ps[:, :nmm], AF.Copy)
                    # ---- deinterleave + clamp ----
                    sr = src[:].rearrange("p (n t) -> p t n", t=3)
                    azc = pc.tile([128, NPP], F32, tag="azc")
                    ayc = pc.tile([128, NPP], F32, tag="ayc")
                    axc = pc.tile([128, NPP], F32, tag="axc")
                    nc.vector.tensor_scalar(azc[:], sr[:, 0], float(-l),
                                            None, ALU.max)
                    nc.vector.tensor_scalar(azc[:], azc[:], float(15 - l),
                                            None, ALU.min)
                    nc.vector.tensor_tensor(ayc[:], sr[:, 1], bndst[:, 1],
                                            ALU.min)
                    nc.vector.tensor_tensor(ayc[:], ayc[:], bndst[:, 0],
                                            ALU.max)
                    nc.vector.tensor_tensor(axc[:], sr[:, 2], bndst[:, 3],
                                            ALU.min)
                    nc.vector.tensor_tensor(axc[:], axc[:], bndst[:, 2],
                                            ALU.max)
                    # ---- tent weights for x, y ----
                    lamx, lamy = [], []
                    for nm, a, row in (("lx", axc, lamx), ("ly", ayc, lamy)):
                        for i, t in enumerate(TAPS):
                            u = pc.tile([128, NPP], F16,
                                        tag="tmpi" if i % 2 == 0 else "tmpb")
                            nc.scalar.activation(u[:], a[:], AF.Abs,
                                                 bias=float(-t), scale=1.0)
                            lt = pc.tile([128, NPP], F16, tag=f"{nm}{i}")
                            nc.scalar.activation(lt[:], u[:], AF.Relu,
                                                 bias=1.0, scale=-1.0)
                            row.append(lt)
                    # ---- gather: separable tent sums ----
                    acc = pc.tile([128, NPP], F16, tag="acc")
                    tmpi = pc.tile([128, NPP], F16, tag="tmpi")
                    tmpb = pc.tile([128, NPP], F16, tag="tmpb")
                    prod = pc.tile([128, NPP], F16, tag="prod")
                    uz = pc.tile([128, NPP], F16, tag="uz")
                    lamz = pc.tile([128, NPP], F16, tag="lz")
                    for iz, sz in enumerate(TAPS):
                        nc.scalar.activation(uz[:], azc[:], AF.Abs,
                                             bias=float(-sz), scale=1.0)
                        nc.scalar.activation(lamz[:], uz[:], AF.Relu,
                                             bias=1.0, scale=-1.0)
                        zi = l - lc0 + 2 + sz
                        for iy, sy in enumerate(TAPS):
                            for ix, sx in enumerate(TAPS):
                                v = win[:, 3 + sy:3 + sy + HG, zi,
                                        2 + sx:2 + sx + W]
                                if ix == 0:
                                    nc.vector.tensor_tensor(
                                        tmpi[:], lamx[0][:], v, ALU.mult)
                                else:
                                    nc.vector.tensor_tensor(
                                        prod[:], lamx[ix][:], v, ALU.mult)
                                    nc.vector.tensor_tensor(
                                        tmpi[:], tmpi[:], prod[:], ALU.add)
                            if iy == 0:
                                nc.vector.tensor_tensor(
                                    tmpb[:], lamy[0][:], tmpi[:], ALU.mult)
                            else:
                                nc.vector.tensor_tensor(
                                    tmpi[:], lamy[iy][:], tmpi[:], ALU.mult)
                                nc.vector.tensor_tensor(
                                    tmpb[:], tmpb[:], tmpi[:], ALU.add)
                        if iz == 0:
                            nc.vector.tensor_tensor(
                                acc[:], lamz[:], tmpb[:], ALU.mult)
                        else:
                            nc.vector.tensor_tensor(
                                tmpb[:], lamz[:], tmpb[:], ALU.mult)
                            nc.vector.tensor_tensor(
                                acc[:], acc[:], tmpb[:], ALU.add)
                    accb = pc.tile([128, HG, 98], F16, tag=f"accb{l % 4}")
                    accb_t[l] = accb
                    nc.vector.memset(
                        accb[:].rearrange("p h x -> p (h x)"), 0.0)
                    nc.vector.tensor_tensor(accb[:, :, 1:97], acc[:],
                                            bndst[:, 4], ALU.mult)
                    if l >= 1:
                        emit_p3(l - 1)

            emit_p3(15)

    nc.finalize()
    return nc


def kernel(x, w_off, w_conv, b_conv):
    global _nc_cache
    import ml_dtypes
    bf16 = ml_dtypes.bfloat16
    f16 = np.float16
    x = np.asarray(x, dtype=np.float32)
    w_off = np.asarray(w_off, dtype=np.float32)
    w_conv = np.asarray(w_conv, dtype=np.float32)
    b_conv = np.asarray(b_conv, dtype=np.float32)

    if _nc_cache is None:
        _nc_cache = build_fused()

    # padded x: depths +-1 (idx=1+l), rows +-4 (idx=4+h), w +-1 (idx=1+w)
    xp = np.zeros((B, C, 18, 104, 98), bf16)
    xph = np.zeros((B, C, L, 104, 98), f16)
    xph[:, :, :, 4:100, 1:97] = x
    xp[:, :, 1:17, 4:100, 1:97] = x

    w3 = w_off.reshape(64, 3, 64, 27)        # [cp, j, ci, tap]
    wct1 = np.ascontiguousarray(
        w_conv.reshape(64, 64, 27).transpose(1, 2, 0)).astype(f16)
    wct = np.concatenate([wct1, wct1], axis=0)
    bct = np.ascontiguousarray(b_conv.reshape(64, 1))

    gxr = np.tile(np.arange(W, dtype=np.float32), HG)
    in_maps = []
    for k in range(NCORES):
        sched = core_sched(k)
        xwin_k = np.ascontiguousarray(
            xph[:, :, :, 12 * k:12 * k + 20, 1:97])
        xa_k = np.empty((B, C, L, 3, 48, 98), bf16)
        ws_k = np.empty((128, L, 3, 27, 64), bf16)
        for l in range(L):
            for seg, ((i0, nr), (s, r0)) in enumerate(zip(SEGS, sched)):
                j, lp = divmod(3 * l + s, 16)
                rb = SEG_RBASE[seg]
                xa_k[:, :, l, :, rb:rb + 3 * nr + 2, :] = \
                    xp[:, :, lp:lp + 3, 4 + r0 - 1:4 + r0 + 3 * nr + 1, :]
                ws_k[:64, l, seg] = w3[:, j].transpose(1, 2, 0).astype(bf16)
                ws_k[64:, l, seg] = ws_k[:64, l, seg]
        ghr = np.repeat(np.arange(HG, dtype=np.float32) + (12 * k - 1), W)
        ymask = ((ghr >= 0) & (ghr <= 95)).astype(np.float32)
        bnds = np.broadcast_to(
            np.stack([-ghr, 95.0 - ghr, -gxr, 95.0 - gxr, ymask])[None],
            (128, 5, NPP)).astype(f16)
        in_maps.append({
            "xwin": xwin_k, "xa": np.ascontiguousarray(xa_k),
            "wsched": np.ascontiguousarray(ws_k),
            "wct": wct, "bct": bct,
            "bnds": np.ascontiguousarray(bnds),
        })
    res = run_bass_kernel_spmd(_nc_cache, in_maps, list(range(NCORES)))
    out = np.empty((B, 64, L, H, W), np.float32)
    for k in range(NCORES):
        out[:, :, :, 12 * k:12 * k + HB, :] = res.results[k]["out"]
    return out


# revision 4
# speedup vs baseline: 1.0123x; 1.0123x over previous
"""DeformConv3D on 8 TRN2 cores — SINGLE fused launch.

Per core (h-band of 12 output rows):
  P1': offset conv computed directly in the torch-contiguous-view
       scrambled arrangement this core's gather needs: for each target
       depth l, 3 fixed segments (15/12/15 source rows) select channel
       group j=(3l+s)//16 and source depth l'=(3l+s)%16 via
       host-permuted input slabs (xa) + host-sliced weights (wsched).
       Output = flat [42*96] per plane; component k of the gather
       coords is the stride-3 slice [3n+k].
  P2: clamp coords on-device, separable 5-tap tent-weight gather (DVE)
  P3: main 3x3x3 conv + bias on the 12-row band (bf16 matmuls)
No host round-trip, no collectives: one launch instead of two.
"""
import sys
import numpy as np

sys.path.insert(0, "/opt/trn_rl_repo")
from concourse import bass, bacc, tile, mybir
from concourse.bass_utils import run_bass_kernel_spmd

F32 = mybir.dt.float32
BF16 = mybir.dt.bfloat16
F16 = mybir.dt.float16
ALU = mybir.AluOpType
AF = mybir.ActivationFunctionType

B, C, L, H, W = 2, 64, 16, 96, 96
NCORES = 8
HB = 12                 # output rows per core
HG = 14                 # band rows (12k-1 .. 12k+12)
NPP = HG * W            # 1344
TAPS = (-2, -1, 0, 1, 2)
SEGS = [(0, 5), (5, 4), (9, 5)]          # (band-row start, n band rows)
SEG_RBASE = (0, 17, 31)                  # xa row offset per segment
SEG_COL0 = (0, 1440, 2592)               # src column offset per segment
# P1' psum chunks per segment: (chunk src-row start, n src rows)
SEG_CHUNKS = ([(0, 5), (5, 5), (10, 5)],
              [(0, 4), (4, 4), (8, 4)],
              [(0, 5), (5, 5), (10, 5)])
LCH = 2                 # target-l chunk size for the gather window

_nc_cache = None


def core_sched(k):
    rows = [12 * k - 1 + i for i in range(HG)]
    sm = []
    for h in rows:
        s = 0 if h < 0 else (2 if h > 95 else h // 32)
        sm.append((s, h - 32 * s))
    sched = []
    for (i0, nr) in SEGS:
        s, m32_0 = sm[i0]
        for t in range(nr):
            assert sm[i0 + t] == (s, m32_0 + t)
        sched.append((s, 3 * m32_0))
    return sched


def build_fused():
    nc = bacc.Bacc("TRN2", target_bir_lowering=False, debug=False,
                   num_devices=NCORES)
    xwin = nc.dram_tensor("xwin", [B, C, L, 20, W], F16,
                          kind="ExternalInput").ap()
    xa = nc.dram_tensor("xa", [B, C, L, 3, 48, 98], BF16,
                        kind="ExternalInput").ap()
    ws01_in = nc.dram_tensor("ws01", [128, L, 3, 9, 64], BF16,
                             kind="ExternalInput").ap()
    ws2_in = nc.dram_tensor("ws2", [128, L, 3, 9, 64], BF16,
                            kind="ExternalInput").ap()
    wct_in = nc.dram_tensor("wct", [128, 27, 64], F16,
                            kind="ExternalInput").ap()
    bct_in = nc.dram_tensor("bct", [64, 1], F32, kind="ExternalInput").ap()
    bnds_in = nc.dram_tensor("bnds", [128, 5, NPP], F16,
                             kind="ExternalInput").ap()
    out_ext = nc.dram_tensor("out", [B, 64, L, HB, W], F32,
                             kind="ExternalOutput").ap()

    # const APs for activation biases (-t for tent taps); 0.0/1.0 built in
    for v in (2.0, -1.0, -2.0):
        t_ = nc.alloc_sbuf_tensor(f"cstb{int(v*10)}", [128, 1], F32)
        nc.gpsimd.memset(t_.ap(), v)
        nc.const_aps.aps[(F32, v)] = t_.ap()
    nc.all_engine_barrier()

    with tile.TileContext(nc) as tc:
        with tc.tile_pool(name="pp", bufs=1) as pp, \
             tc.tile_pool(name="pa", bufs=2) as pa, \
             tc.tile_pool(name="pc", bufs=1) as pc, \
             tc.tile_pool(name="psp", bufs=3, space="PSUM") as psp:
            bndst = pp.tile([128, 5, NPP], F16)
            nc.sync.dma_start(bndst[:], bnds_in)
            win = pp.tile([128, 20, LCH + 4, 100], F16)
            wctt = pp.tile([128, 27, 64], F16)
            nc.sync.dma_start(wctt[:], wct_in)
            bctt = pp.tile([64, 1], F32)
            nc.sync.dma_start(bctt[:], bct_in)
            accb_t = {}

            def emit_p3(lp):
                taps = [t for t in range(27)
                        if 0 <= lp + t // 9 - 1 <= 15]
                for b in range(B):
                    for hc0, hcn in ((0, 5), (5, 5), (10, 2)):
                        nmm = hcn * 96
                        ps = psp.tile([64, 480], F32, tag="ps3")
                        for ti, t in enumerate(taps):
                            dz, rem = divmod(t, 9)
                            dy, dx = divmod(rem, 3)
                            rhs = accb_t[lp + dz - 1][
                                64 * b:64 * b + 64,
                                hc0 + dy:hc0 + dy + hcn, dx:dx + 96]
                            nc.tensor.matmul(
                                ps[:, :nmm],
                                wctt[64 * b:64 * b + 64, t, :], rhs,
                                start=(ti == 0), stop=(ti == len(taps) - 1))
                        ob = pc.tile([64, 480], F32, tag="ob")
                        nc.scalar.activation(ob[:, :nmm], ps[:, :nmm],
                                             AF.Identity, bias=bctt[:],
                                             scale=1.0)
                        nc.sync.dma_start(
                            out_ext[b, :, lp, hc0:hc0 + hcn, :]
                            .rearrange("m h x -> m (h x)"),
                            ob[:, :nmm])

            for lc0 in range(0, L, LCH):
                # (re)load gather window depths lc0-2 .. lc0+LCH+1
                nc.vector.memset(
                    win[:].rearrange("p y z x -> p (y z x)"), 0.0)
                for b in range(B):
                    for z in range(lc0 - 2, lc0 + LCH + 2):
                        if 0 <= z < L:
                            nc.sync.dma_start(
                                win[64 * b:64 * b + 64, :, z - lc0 + 2, 2:98],
                                xwin[b, :, z])
                for l in range(lc0, lc0 + LCH):
                    # ---- P1': scrambled offset conv ----
                    xa01_0 = pa.tile([128, 48, 98], BF16, tag="xa01_0")
                    xa01_1 = pa.tile([128, 48, 98], BF16, tag="xa01_1")
                    xa01 = [xa01_0, xa01_1]
                    xa2 = pa.tile([128, 48, 98], BF16, tag="xa2")
                    for b in range(B):
                        for dz in range(2):
                            nc.sync.dma_start(
                                xa01[b][64 * dz:64 * dz + 64], xa[b, :, l, dz])
                        nc.sync.dma_start(
                            xa2[64 * b:64 * b + 64], xa[b, :, l, 2])
                    ws01t = pa.tile([128, 3, 9, 64], BF16, tag="ws01")
                    nc.sync.dma_start(ws01t[:], ws01_in[:, l])
                    ws2t = pa.tile([128, 3, 9, 64], BF16, tag="ws2")
                    nc.sync.dma_start(ws2t[:], ws2_in[:, l])
                    src = pa.tile([128, 4032], F16, tag="src")
                    for b in range(B):
                        for seg in range(3):
                            rb = SEG_RBASE[seg]
                            for (cr0, cn) in SEG_CHUNKS[seg]:
                                nmm = cn * 96
                                ps = psp.tile([64, 480], F32, tag="ps1")
                                for q in range(9):
                                    dy, dx = divmod(q, 3)
                                    rhs = xa01[b][:,
                                                  rb + cr0 + dy:rb + cr0 + dy + cn,
                                                  dx:dx + 96]
                                    nc.tensor.matmul(
                                        ps[:, :nmm], ws01t[:, seg, q, :], rhs,
                                        start=(q == 0), stop=False)
                                for q in range(9):
                                    dy, dx = divmod(q, 3)
                                    rhs = xa2[64 * b:64 * b + 64,
                                              rb + cr0 + dy:rb + cr0 + dy + cn,
                                              dx:dx + 96]
                                    nc.tensor.matmul(
                                        ps[:, :nmm],
                                        ws2t[64 * b:64 * b + 64, seg, q, :],
                                        rhs, start=False, stop=(q == 8))
                                col0 = SEG_COL0[seg] + cr0 * 96
                                nc.scalar.activation(
                                    src[64 * b:64 * b + 64,
                                        col0:col0 + nmm],
                                    ps[:, :nmm], AF.Copy)
                    # ---- deinterleave + clamp ----
                    sr = src[:].rearrange("p (n t) -> p t n", t=3)
                    azc = pc.tile([128, NPP], F32, tag="azc")
                    ayc = pc.tile([128, NPP], F32, tag="ayc")
                    axc = pc.tile([128, NPP], F32, tag="axc")
                    nc.vector.tensor_scalar(azc[:], sr[:, 0], float(-l),
                                            None, ALU.max)
                    nc.vector.tensor_scalar(azc[:], azc[:], float(15 - l),
                                            None, ALU.min)
                    nc.vector.tensor_tensor(ayc[:], sr[:, 1], bndst[:, 1],
                                            ALU.min)
                    nc.vector.tensor_tensor(ayc[:], ayc[:], bndst[:, 0],
                                            ALU.max)
                    nc.vector.tensor_tensor(axc[:], sr[:, 2], bndst[:, 3],
                                            ALU.min)
                    nc.vector.tensor_tensor(axc[:], axc[:], bndst[:, 2],
                                            ALU.max)
                    # ---- tent weights for x, y ----
                    lamx, lamy = [], []
                    for nm, a, row in (("lx", axc, lamx), ("ly", ayc, lamy)):
                        for i, t in enumerate(TAPS):
                            u = pc.tile([128, NPP], F16,
                                        tag="tmpi" if i % 2 == 0 else "tmpb")
                            nc.scalar.activation(u[:], a[:], AF.Abs,
                                                 bias=float(-t), scale=1.0)
                            lt = pc.tile([128, NPP], F16, tag=f"{nm}{i}")
                            nc.scalar.activation(lt[:], u[:], AF.Relu,
                                                 bias=1.0, scale=-1.0)
                            row.append(lt)
                    # ---- gather: separable tent sums ----
                    acc = pc.tile([128, NPP], F16, tag="acc")
                    tmpi = pc.tile([128, NPP], F16, tag="tmpi")
                    tmpb = pc.tile([128, NPP], F16, tag="tmpb")
                    prod = pc.tile([128, NPP], F16, tag="prod")
                    uz = pc.tile([128, NPP], F16, tag="uz")
                    lamz = pc.tile([128, NPP], F16, tag="lz")
                    for iz, sz in enumerate(TAPS):
                        nc.scalar.activation(uz[:], azc[:], AF.Abs,
                                             bias=float(-sz), scale=1.0)
                        nc.scalar.activation(lamz[:], uz[:], AF.Relu,
                                             bias=1.0, scale=-1.0)
                        zi = l - lc0 + 2 + sz
                        for iy, sy in enumerate(TAPS):
                            for ix, sx in enumerate(TAPS):
                                v = win[:, 3 + sy:3 + sy + HG, zi,
                                        2 + sx:2 + sx + W]
                                if ix == 0:
                                    nc.vector.tensor_tensor(
                                        tmpi[:], lamx[0][:], v, ALU.mult)
                                else:
                                    nc.vector.tensor_tensor(
                                        prod[:], lamx[ix][:], v, ALU.mult)
                                    nc.vector.tensor_tensor(
                                        tmpi[:], tmpi[:], prod[:], ALU.add)
                            if iy == 0:
                                nc.vector.tensor_tensor(
                                    tmpb[:], lamy[0][:], tmpi[:], ALU.mult)
                            else:
                                nc.vector.tensor_tensor(
                                    tmpi[:], lamy[iy][:], tmpi[:], ALU.mult)
                                nc.vector.tensor_tensor(
                                    tmpb[:], tmpb[:], tmpi[:], ALU.add)
                        if iz == 0:
                            nc.vector.tensor_tensor(
                                acc[:], lamz[:], tmpb[:], ALU.mult)
                        else:
                            nc.vector.tensor_tensor(
                                tmpb[:], lamz[:], tmpb[:], ALU.mult)
                            nc.vector.tensor_tensor(
                                acc[:], acc[:], tmpb[:], ALU.add)
                    accb = pc.tile([128, HG, 98], F16, tag=f"accb{l % 4}")
                    accb_t[l] = accb
                    nc.vector.memset(
                        accb[:].rearrange("p h x -> p (h x)"), 0.0)
                    nc.vector.tensor_tensor(accb[:, :, 1:97], acc[:],
                                            bndst[:, 4], ALU.mult)
                    if l >= 1:
                        emit_p3(l - 1)

            emit_p3(15)

    nc.finalize()
    return nc


def kernel(x, w_off, w_conv, b_conv):
    global _nc_cache
    import ml_dtypes
    bf16 = ml_dtypes.bfloat16
    f16 = np.float16
    x = np.asarray(x, dtype=np.float32)
    w_off = np.asarray(w_off, dtype=np.float32)
    w_conv = np.asarray(w_conv, dtype=np.float32)
    b_conv = np.asarray(b_conv, dtype=np.float32)

    if _nc_cache is None:
        _nc_cache = build_fused()

    # padded x: depths +-1 (idx=1+l), rows +-4 (idx=4+h), w +-1 (idx=1+w)
    xp = np.zeros((B, C, 18, 104, 98), bf16)
    xph = np.zeros((B, C, L, 104, 98), f16)
    xph[:, :, :, 4:100, 1:97] = x
    xp[:, :, 1:17, 4:100, 1:97] = x

    w3 = w_off.reshape(64, 3, 64, 27)        # [cp, j, ci, tap]
    wct1 = np.ascontiguousarray(
        w_conv.reshape(64, 64, 27).transpose(1, 2, 0)).astype(f16)
    wct = np.concatenate([wct1, wct1], axis=0)
    bct = np.ascontiguousarray(b_conv.reshape(64, 1))

    gxr = np.tile(np.arange(W, dtype=np.float32), HG)
    in_maps = []
    for k in range(NCORES):
        sched = core_sched(k)
        xwin_k = np.ascontiguousarray(
            xph[:, :, :, 12 * k:12 * k + 20, 1:97])
        xa_k = np.empty((B, C, L, 3, 48, 98), bf16)
        ws01_k = np.empty((128, L, 3, 9, 64), bf16)
        ws2_k = np.empty((128, L, 3, 9, 64), bf16)
        for l in range(L):
            for seg, ((i0, nr), (s, r0)) in enumerate(zip(SEGS, sched)):
                j, lp = divmod(3 * l + s, 16)
                rb = SEG_RBASE[seg]
                xa_k[:, :, l, :, rb:rb + 3 * nr + 2, :] = \
                    xp[:, :, lp:lp + 3, 4 + r0 - 1:4 + r0 + 3 * nr + 1, :]
                wj = w3[:, j].transpose(1, 2, 0).astype(bf16)  # [ci, 27, cp]
                ws01_k[:64, l, seg] = wj[:, 0:9]
                ws01_k[64:, l, seg] = wj[:, 9:18]
                ws2_k[:64, l, seg] = wj[:, 18:27]
                ws2_k[64:, l, seg] = wj[:, 18:27]
        ghr = np.repeat(np.arange(HG, dtype=np.float32) + (12 * k - 1), W)
        ymask = ((ghr >= 0) & (ghr <= 95)).astype(np.float32)
        bnds = np.broadcast_to(
            np.stack([-ghr, 95.0 - ghr, -gxr, 95.0 - gxr, ymask])[None],
            (128, 5, NPP)).astype(f16)
        in_maps.append({
            "xwin": xwin_k, "xa": np.ascontiguousarray(xa_k),
            "ws01": np.ascontiguousarray(ws01_k),
            "ws2": np.ascontiguousarray(ws2_k),
            "wct": wct, "bct": bct,
            "bnds": np.ascontiguousarray(bnds),
        })
    res = run_bass_kernel_spmd(_nc_cache, in_maps, list(range(NCORES)))
    out = np.empty((B, 64, L, H, W), np.float32)
    for k in range(NCORES):
        out[:, :, :, 12 * k:12 * k + HB, :] = res.results[k]["out"]
    return out
